# revision 25
# baseline (speedup 1.0000x reference)
"""BiGRU Trainium2 kernel: B=64, T=512, D=256, U=512, 8 NeuronCores.

Sharding: 8 cores = 2 directions x 4 batch-quarters (B_local=16).
The GRU recurrence is latency-bound (T sequential steps); each core runs one
chain for (direction, batch-quarter) with a packed layout:
  partition p = 32*g + b   (g = U-block 0..3, b = local batch 0..15)
  column   c = offset within U-block (0..127);  u = 128*g + c

Gate matmuls are column-tiled across the 4 PE column groups: each group
streams its own slice of Wh (host-prepacked, bf16) against the stationary
transposed state hT (bf16).  Input projections gx(t+1) run just-in-time into
the PSUM banks so the recurrent matmuls accumulate on top of them.
"""

import sys
import os

for _p in ("/opt/trn_rl_repo",):
    if os.path.isdir(_p) and _p not in sys.path:
        sys.path.insert(0, _p)

import numpy as np
from contextlib import ExitStack

import concourse.bass as bass
import concourse.bacc as bacc
import concourse.tile as tile
from concourse import mybir
from concourse.bass_utils import run_bass_kernel_spmd

try:
    from ml_dtypes import bfloat16
except ImportError:  # pragma: no cover
    import jax.numpy as _jnp

    bfloat16 = _jnp.bfloat16

B, T, D, U = 64, 512, 256, 512
NCORES = 8
BL = B // 4  # 16 local batch per core (4 batch quarters x 2 directions)
NG = 4  # U blocks of 128
KC_H = 4  # contraction chunks over U (512/128)
KC_X = 2  # contraction chunks over D (256/128)

F32 = mybir.dt.float32
BF16 = mybir.dt.bfloat16

OUT_BLOCK = 8  # steps per output DMA flush


def build_program(t_steps=T, with_bias=False):
    """Builds the SPMD Bass program (identical for all cores)."""
    # Bacc (not plain Bass): its compile pipeline splits multi-sem waits into
    # EventSemaphore instructions — TRN2 instructions hold at most one wait.
    nc = bacc.Bacc(None, target_bir_lowering=False)

    xT = nc.dram_tensor("xT", [128, KC_X, t_steps, BL], BF16, kind="ExternalInput")
    wh_ur = nc.dram_tensor("wh_ur", [128, KC_H, NG, 256], BF16, kind="ExternalInput")
    wh_hh = nc.dram_tensor("wh_hh", [128, KC_H, NG, 128], BF16, kind="ExternalInput")
    wx_all = nc.dram_tensor("wx_all", [128, KC_X, NG, 384], BF16, kind="ExternalInput")
    ident = nc.dram_tensor("ident", [128, 128], F32, kind="ExternalInput")
    zrow = nc.dram_tensor("zrow", [1, 384], BF16, kind="ExternalInput")
    ones_row = nc.dram_tensor("ones_row", [1, 128], BF16, kind="ExternalInput")
    if with_bias:
        bias_ur = nc.dram_tensor("bias_ur", [1, NG, 256], F32, kind="ExternalInput")
        bias_hh = nc.dram_tensor("bias_hh", [1, NG, 128], F32, kind="ExternalInput")
    out = nc.dram_tensor("out_packed", [128, t_steps, 128], F32, kind="ExternalOutput")

    with tile.TileContext(nc) as tc, ExitStack() as ctx:
        singles = ctx.enter_context(tc.tile_pool(name="singles", bufs=1))
        hT_pool = ctx.enter_context(tc.tile_pool(name="hT", bufs=3))
        tT_pool = ctx.enter_context(tc.tile_pool(name="tT", bufs=2))
        gates = ctx.enter_context(tc.tile_pool(name="gates", bufs=3))
        stage_pool = ctx.enter_context(tc.tile_pool(name="stage", bufs=2))
        ps_ur = ctx.enter_context(tc.tile_pool(name="ps_ur", bufs=2, space="PSUM"))
        ps_hh = ctx.enter_context(tc.tile_pool(name="ps_hh", bufs=2, space="PSUM"))
        ps_rT = ctx.enter_context(tc.tile_pool(name="ps_rT", bufs=2, space="PSUM"))
        ps_hT = ctx.enter_context(tc.tile_pool(name="ps_hT", bufs=2, space="PSUM"))

        # --- resident inputs ---
        xT_sb = singles.tile([128, KC_X, t_steps, BL], BF16)
        wh_ur_sb = singles.tile([128, KC_H, NG, 256], BF16)
        wh_hh_sb = singles.tile([128, KC_H, NG, 128], BF16)
        wx_sb = singles.tile([128, KC_X, NG, 384], BF16)
        ident_sb = singles.tile([128, 128], F32)
        nc.sync.dma_start(out=xT_sb[:], in_=xT[:])
        nc.sync.dma_start(out=wh_ur_sb[:], in_=wh_ur[:])
        nc.sync.dma_start(out=wh_hh_sb[:], in_=wh_hh[:])
        nc.sync.dma_start(out=wx_sb[:], in_=wx_all[:])
        nc.sync.dma_start(out=ident_sb[:], in_=ident[:])
        zrow_sb = singles.tile([1, 384], BF16)
        ones_sb = singles.tile([1, 128], BF16)
        nc.sync.dma_start(out=zrow_sb[:], in_=zrow[:])
        nc.sync.dma_start(out=ones_sb[:], in_=ones_row[:])
        if with_bias:
            bias_ur_sb = singles.tile([1, NG, 256], F32)
            bias_hh_sb = singles.tile([1, NG, 128], F32)
            nc.sync.dma_start(out=bias_ur_sb[:], in_=bias_ur[:])
            nc.sync.dma_start(out=bias_hh_sb[:], in_=bias_hh[:])

        # --- initial state (h = 0) ---
        h0 = singles.tile([128, 128], F32)
        hT0 = singles.tile([128, 128], BF16)
        nc.vector.memset(h0[:], 0.0)
        nc.vector.memset(hT0[:], 0.0)

        # Pre-consume ident on PE: transpose-mode matmuls lower to the LDW
        # struct which holds only ONE sync wait; without this the first real
        # transpose would need both its data wait and the ident-DMA wait.
        warm = ps_rT.tile([128, 128], F32, tag="rT")
        nc.tensor.transpose(warm[:], ident_sb[:], ident_sb[:])

        def emit_gx(t):
            """Input-projection matmuls for step t -> fresh psum tiles."""
            pu = ps_ur.tile([128, 256], F32)
            ph = ps_hh.tile([128, 128], F32)
            # Broadcast-init both tiles (K=1 matmul of a zeros row): zeroes
            # every partition incl. unused lanes, so downstream full-width
            # reads are always on initialized data.
            nc.tensor.matmul(
                pu[:, :],
                lhsT=ones_sb[:],
                rhs=zrow_sb[:, 0:256],
                start=True,
                stop=False,
                skip_group_check=True,
            )
            nc.tensor.matmul(
                ph[:, :],
                lhsT=ones_sb[:],
                rhs=zrow_sb[:, 256:384],
                start=True,
                stop=False,
                skip_group_check=True,
            )
            if with_bias:
                for g in range(4):
                    nc.tensor.matmul(
                        pu[32 * g : 32 * g + 16, :],
                        lhsT=ones_sb[:, 0:16],
                        rhs=bias_ur_sb[:, g, :],
                        start=False,
                        stop=False,
                        tile_position=(0, 32 * g),
                        skip_group_check=True,
                    )
                    nc.tensor.matmul(
                        ph[32 * g : 32 * g + 16, :],
                        lhsT=ones_sb[:, 0:16],
                        rhs=bias_hh_sb[:, g, :],
                        start=False,
                        stop=False,
                        tile_position=(0, 32 * g),
                        skip_group_check=True,
                    )
            for kc in range(KC_X):
                for g in range(4):
                    st = xT_sb[:, kc, t, :]
                    nc.tensor.matmul(
                        pu[32 * g : 32 * g + 16, :],
                        lhsT=st,
                        rhs=wx_sb[:, kc, g, 0:256],
                        start=False,
                        stop=False,
                        tile_position=(0, 32 * g),
                        skip_group_check=True,
                    )
                    nc.tensor.matmul(
                        ph[32 * g : 32 * g + 16, :],
                        lhsT=st,
                        rhs=wx_sb[:, kc, g, 256:384],
                        start=False,
                        stop=False,
                        tile_position=(0, 32 * g),
                        skip_group_check=True,
                    )
            return pu, ph

        pu_cur, ph_cur = emit_gx(0)

        h_prev = h0  # f32 packed [128,128] (AP-able)
        h_prev_ap = h0[:]
        hT_prev = hT0
        stage_cur = None

        for t in range(t_steps):
            if t % OUT_BLOCK == 0:
                stage_cur = stage_pool.tile([128, OUT_BLOCK, 128], F32)

            # ---- gx for t+1 (fills PE gaps while step-t chain runs) ----
            if t + 1 < t_steps:
                pu_nxt, ph_nxt = emit_gx(t + 1)
            else:
                pu_nxt = ph_nxt = None

            # ---- recurrent matmuls: gates u,r ----
            for kc in range(KC_H):
                for g in range(4):
                    nc.tensor.matmul(
                        pu_cur[32 * g : 32 * g + 16, :],
                        lhsT=hT_prev[:, 32 * kc : 32 * kc + 16],
                        rhs=wh_ur_sb[:, kc, g, :],
                        start=False,
                        stop=(kc == KC_H - 1) and (g == 3),
                        tile_position=(0, 32 * g),
                        skip_group_check=True,
                    )

            # ---- sigmoid(r), sigmoid(u) ----
            r_s = gates.tile([128, 128], F32, tag="r_s")
            u_s = gates.tile([128, 128], F32, tag="u_s")
            nc.scalar.activation(
                r_s[:], pu_cur[:, 128:256], mybir.ActivationFunctionType.Sigmoid
            )
            nc.scalar.activation(
                u_s[:], pu_cur[:, 0:128], mybir.ActivationFunctionType.Sigmoid
            )

            # ---- transpose r, tT = rT * hT ----
            rT = ps_rT.tile([128, 128], F32)
            nc.tensor.transpose(rT[:], r_s[:], ident_sb[:])
            tT = tT_pool.tile([128, 128], BF16)
            nc.vector.tensor_mul(tT[:], rT[:], hT_prev[:])

            # ---- candidate matmul ----
            for kc in range(KC_H):
                for g in range(4):
                    nc.tensor.matmul(
                        ph_cur[32 * g : 32 * g + 16, :],
                        lhsT=tT[:, 32 * kc : 32 * kc + 16],
                        rhs=wh_hh_sb[:, kc, g, :],
                        start=False,
                        stop=(kc == KC_H - 1) and (g == 3),
                        tile_position=(0, 32 * g),
                        skip_group_check=True,
                    )

            # ---- tanh, blend: h_new = u*hh + (1-u)*h = u*hh - (u-1)*h ----
            hh_s = gates.tile([128, 128], F32, tag="hh_s")
            nc.scalar.activation(
                hh_s[:], ph_cur[:], mybir.ActivationFunctionType.Tanh
            )
            w_s = gates.tile([128, 128], F32, tag="w_s")
            nc.vector.scalar_tensor_tensor(
                w_s[:],
                in0=u_s[:],
                scalar=1.0,
                in1=h_prev_ap,
                op0=mybir.AluOpType.subtract,
                op1=mybir.AluOpType.mult,
            )
            v_s = gates.tile([128, 128], F32, tag="v_s")
            nc.vector.tensor_mul(v_s[:], u_s[:], hh_s[:])
            h_new_ap = stage_cur[:, t % OUT_BLOCK, :]
            nc.vector.tensor_sub(h_new_ap, v_s[:], w_s[:])

            # ---- transpose h_new -> hT (bf16) for next step ----
            hT_new = hT_pool.tile([128, 128], BF16)
            pT = ps_hT.tile([128, 128], F32)
            nc.tensor.transpose(pT[:], h_new_ap, ident_sb[:])
            nc.scalar.copy(out=hT_new[:], in_=pT[:])

            # ---- flush output block ----
            if (t + 1) % OUT_BLOCK == 0 or t == t_steps - 1:
                n = t % OUT_BLOCK + 1
                t0 = t - n + 1
                nc.sync.dma_start(
                    out=out[:, t0 : t0 + n, :],
                    in_=stage_cur[:, 0:n, :],
                )

            h_prev = stage_cur
            h_prev_ap = h_new_ap
            hT_prev = hT_new
            pu_cur, ph_cur = pu_nxt, ph_nxt

    # Bacc.finalize runs the compile pipeline (wait splitting, register
    # allocation). run_bass_via_pjrt serializes nc as-is, so do it here.
    nc.finalize()
    return nc


# ---------------------------------------------------------------------------
# Host-side packing / unpacking
# ---------------------------------------------------------------------------


def _pack_x(xc):
    """xc [BL, t, 256] (already direction-adjusted) -> [128, 2, t, BL] bf16."""
    t = xc.shape[1]
    a = np.ascontiguousarray(xc.transpose(2, 1, 0))  # [256, t, BL]
    a = a.reshape(KC_X, 128, t, BL).transpose(1, 0, 2, 3)
    return np.ascontiguousarray(a).astype(bfloat16)


def _pack_wh(wh):
    u = wh[:, 0:512].reshape(512, 4, 128)
    r = wh[:, 512:1024].reshape(512, 4, 128)
    hh = wh[:, 1024:1536].reshape(512, 4, 128)
    ur = np.concatenate([u, r], axis=2)  # [512, 4, 256]
    ur = ur.reshape(KC_H, 128, 4, 256).transpose(1, 0, 2, 3)
    hh = hh.reshape(KC_H, 128, 4, 128).transpose(1, 0, 2, 3)
    return (
        np.ascontiguousarray(ur).astype(bfloat16),
        np.ascontiguousarray(hh).astype(bfloat16),
    )


def _pack_wx(wx):
    u = wx[:, 0:512].reshape(256, 4, 128)
    r = wx[:, 512:1024].reshape(256, 4, 128)
    hh = wx[:, 1024:1536].reshape(256, 4, 128)
    a = np.concatenate([u, r, hh], axis=2)  # [256, 4, 384]
    a = a.reshape(KC_X, 128, 4, 384).transpose(1, 0, 2, 3)
    return np.ascontiguousarray(a).astype(bfloat16)


def _pack_bias(b):
    u = b[0:512].reshape(4, 128)
    r = b[512:1024].reshape(4, 128)
    hh = b[1024:1536].reshape(4, 128)
    ur = np.concatenate([u, r], axis=1)[None].astype(np.float32)  # [1, 4, 256]
    return np.ascontiguousarray(ur), np.ascontiguousarray(
        hh[None].astype(np.float32)
    )


def make_in_maps(x, Wx_f, Wh_f, b_f, Wx_b, Wh_b, b_b, t_steps=T, with_bias=False):
    x = np.asarray(x, dtype=np.float32)
    ident = np.eye(128, dtype=np.float32)
    per_dir = {}
    for d, (wx, wh, bb) in enumerate(
        [(Wx_f, Wh_f, b_f), (Wx_b, Wh_b, b_b)]
    ):
        ur, hh = _pack_wh(np.asarray(wh, np.float32))
        wxp = _pack_wx(np.asarray(wx, np.float32))
        ent = {"wh_ur": ur, "wh_hh": hh, "wx_all": wxp}
        if with_bias:
            bur, bhh = _pack_bias(np.asarray(bb, np.float32))
            ent.update(bias_ur=bur, bias_hh=bhh)
        per_dir[d] = ent

    in_maps = []
    for c in range(NCORES):
        d, q = divmod(c, 4)
        xc = x[q * BL : (q + 1) * BL, :t_steps]
        if d == 1:
            xc = xc[:, ::-1]
        m = {
            "xT": _pack_x(xc),
            "ident": ident,
            "zrow": np.zeros((1, 384), dtype=bfloat16),
            "ones_row": np.ones((1, 128), dtype=bfloat16),
        }
        m.update(per_dir[d])
        in_maps.append(m)
    return in_maps


def unpack_outputs(results, t_steps=T):
    out = np.empty((B, t_steps, 2 * U), dtype=np.float32)
    for c in range(NCORES):
        d, q = divmod(c, 4)
        r = results[c]["out_packed"].reshape(4, 32, t_steps, 128)[:, 0:BL]
        out[q * BL : (q + 1) * BL, :, d * U : (d + 1) * U] = (
            r.transpose(1, 2, 0, 3).reshape(BL, t_steps, U)
        )
    return out


_CACHE = {}


def kernel(x, Wx_f, Wh_f, b_f, Wx_b, Wh_b, b_b):
    with_bias = bool(
        np.any(np.asarray(b_f) != 0) or np.any(np.asarray(b_b) != 0)
    )
    key = ("prog", T, with_bias)
    if key not in _CACHE:
        _CACHE[key] = build_program(T, with_bias)
    nc = _CACHE[key]
    in_maps = make_in_maps(
        x, Wx_f, Wh_f, b_f, Wx_b, Wh_b, b_b, T, with_bias
    )
    res = run_bass_kernel_spmd(nc, in_maps, list(range(NCORES)))
    return unpack_outputs(res.results, T)


if __name__ == "__main__":
    mode = sys.argv[1] if len(sys.argv) > 1 else "sim"
    if mode == "sim":
        # Small-T single-core simulation vs numpy GRU.
        ts = int(sys.argv[2]) if len(sys.argv) > 2 else 8
        rng = np.random.default_rng(0)
        x = rng.standard_normal((B, ts, D), dtype=np.float32)
        Wx = (rng.standard_normal((D, 3 * U), dtype=np.float32) / np.sqrt(D)).astype(
            np.float32
        )
        Wh = (rng.standard_normal((U, 3 * U), dtype=np.float32) / np.sqrt(U)).astype(
            np.float32
        )
        bz = np.zeros(3 * U, np.float32)

        nc = build_program(ts, with_bias=False)
        in_maps = make_in_maps(x, Wx, Wh, bz, Wx, Wh, bz, ts, False)

        from concourse.bass_interp import MultiCoreSim

        sim = MultiCoreSim(nc, 1)
        for k, v in in_maps[0].items():
            sim.cores[0].tensor(k)[:] = v
        sim.simulate()
        got = sim.cores[0].tensor("out_packed")  # [128, ts, 128]
        got = (
            got.reshape(4, 32, ts, 128)[:, 0:BL]
            .transpose(1, 2, 0, 3)
            .reshape(BL, ts, U)
        )

        # numpy reference GRU (forward, batch quarter 0)
        h = np.zeros((BL, U), np.float32)
        exp = np.zeros((BL, ts, U), np.float32)
        xs = x[0:BL, :ts].astype(np.float32)
        for t in range(ts):
            gx = xs[:, t] @ Wx
            pu = gx[:, :U] + h @ Wh[:, :U]
            pr = gx[:, U : 2 * U] + h @ Wh[:, U : 2 * U]
            u = 1 / (1 + np.exp(-pu))
            r = 1 / (1 + np.exp(-pr))
            hh = np.tanh(gx[:, 2 * U :] + (r * h) @ Wh[:, 2 * U :])
            h = (1 - u) * h + u * hh
            exp[:, t] = h
        err = np.abs(got - exp)
        denom = max(1e-6, np.abs(exp).max())
        print("max abs err:", err.max(), " rel:", err.max() / denom)
        print("sample got:", got[0, -1, :4], " exp:", exp[0, -1, :4])
    else:
        print("unknown mode", mode)


# revision 31
# speedup vs baseline: 1.0477x; 1.0477x over previous
"""BiGRU Trainium2 kernel: B=64, T=512, D=256, U=512, 8 NeuronCores.

Sharding: 8 cores = 2 directions x 4 batch-quarters (B_local=16).
The GRU recurrence is latency-bound (T sequential steps); each core runs one
chain for (direction, batch-quarter) with a packed layout:
  partition p = 32*g + b   (g = U-block 0..3, b = local batch 0..15)
  column   c = offset within U-block (0..127);  u = 128*g + c

Gate matmuls are column-tiled across the 4 PE column groups: each group
streams its own slice of Wh (host-prepacked, bf16) against the stationary
transposed state hT (bf16).  Input projections gx(t+1) run just-in-time into
the PSUM banks so the recurrent matmuls accumulate on top of them.
"""

import sys
import os

for _p in ("/opt/trn_rl_repo",):
    if os.path.isdir(_p) and _p not in sys.path:
        sys.path.insert(0, _p)

import numpy as np
from contextlib import ExitStack

import concourse.bass as bass
import concourse.bacc as bacc
import concourse.tile as tile
from concourse import mybir
from concourse.bass_utils import run_bass_kernel_spmd

try:
    from ml_dtypes import bfloat16
except ImportError:  # pragma: no cover
    import jax.numpy as _jnp

    bfloat16 = _jnp.bfloat16

B, T, D, U = 64, 512, 256, 512
NCORES = 8
BL = B // 4  # 16 local batch per core (4 batch quarters x 2 directions)
NG = 4  # U blocks of 128
KC_H = 4  # contraction chunks over U (512/128)
KC_X = 2  # contraction chunks over D (256/128)

F32 = mybir.dt.float32
BF16 = mybir.dt.bfloat16

OUT_BLOCK = 8  # steps per output DMA flush


def build_program(t_steps=T, with_bias=False):
    """Builds the SPMD Bass program (identical for all cores)."""
    # Bacc (not plain Bass): its compile pipeline splits multi-sem waits into
    # EventSemaphore instructions — TRN2 instructions hold at most one wait.
    nc = bacc.Bacc(None, target_bir_lowering=False)

    xT = nc.dram_tensor("xT", [128, KC_X, t_steps, BL], BF16, kind="ExternalInput")
    wh_ur = nc.dram_tensor("wh_ur", [128, KC_H, NG, 256], BF16, kind="ExternalInput")
    wh_hh = nc.dram_tensor("wh_hh", [128, KC_H, NG, 128], BF16, kind="ExternalInput")
    wx_all = nc.dram_tensor("wx_all", [128, KC_X, NG, 384], BF16, kind="ExternalInput")
    ident = nc.dram_tensor("ident", [128, 128], F32, kind="ExternalInput")
    ident16 = nc.dram_tensor("ident16", [128, 128], BF16, kind="ExternalInput")
    zrow = nc.dram_tensor("zrow", [1, 384], BF16, kind="ExternalInput")
    ones_row = nc.dram_tensor("ones_row", [1, 128], BF16, kind="ExternalInput")
    if with_bias:
        bias_ur = nc.dram_tensor("bias_ur", [1, NG, 256], F32, kind="ExternalInput")
        bias_hh = nc.dram_tensor("bias_hh", [1, NG, 128], F32, kind="ExternalInput")
    out = nc.dram_tensor("out_packed", [128, t_steps, 128], F32, kind="ExternalOutput")

    with tile.TileContext(nc) as tc, ExitStack() as ctx:
        singles = ctx.enter_context(tc.tile_pool(name="singles", bufs=1))
        hT_pool = ctx.enter_context(tc.tile_pool(name="hT", bufs=3))
        tT_pool = ctx.enter_context(tc.tile_pool(name="tT", bufs=2))
        gates = ctx.enter_context(tc.tile_pool(name="gates", bufs=3))
        stage_pool = ctx.enter_context(tc.tile_pool(name="stage", bufs=2))
        ps_ur = ctx.enter_context(tc.tile_pool(name="ps_ur", bufs=2, space="PSUM"))
        ps_hh = ctx.enter_context(tc.tile_pool(name="ps_hh", bufs=2, space="PSUM"))
        ps_rT = ctx.enter_context(tc.tile_pool(name="ps_rT", bufs=2, space="PSUM"))
        ps_hT = ctx.enter_context(tc.tile_pool(name="ps_hT", bufs=2, space="PSUM"))

        # --- resident inputs ---
        xT_sb = singles.tile([128, KC_X, t_steps, BL], BF16)
        wh_ur_sb = singles.tile([128, KC_H, NG, 256], BF16)
        wh_hh_sb = singles.tile([128, KC_H, NG, 128], BF16)
        wx_sb = singles.tile([128, KC_X, NG, 384], BF16)
        ident_sb = singles.tile([128, 128], F32)
        ident16_sb = singles.tile([128, 128], BF16)
        nc.sync.dma_start(out=ident16_sb[:], in_=ident16[:])
        nc.sync.dma_start(out=xT_sb[:], in_=xT[:])
        nc.sync.dma_start(out=wh_ur_sb[:], in_=wh_ur[:])
        nc.sync.dma_start(out=wh_hh_sb[:], in_=wh_hh[:])
        nc.sync.dma_start(out=wx_sb[:], in_=wx_all[:])
        nc.sync.dma_start(out=ident_sb[:], in_=ident[:])
        zrow_sb = singles.tile([1, 384], BF16)
        ones_sb = singles.tile([1, 128], BF16)
        nc.sync.dma_start(out=zrow_sb[:], in_=zrow[:])
        nc.sync.dma_start(out=ones_sb[:], in_=ones_row[:])
        if with_bias:
            bias_ur_sb = singles.tile([1, NG, 256], F32)
            bias_hh_sb = singles.tile([1, NG, 128], F32)
            nc.sync.dma_start(out=bias_ur_sb[:], in_=bias_ur[:])
            nc.sync.dma_start(out=bias_hh_sb[:], in_=bias_hh[:])

        # --- initial state (h = 0) ---
        h0 = singles.tile([128, 128], F32)
        hT0 = singles.tile([128, 128], BF16)
        nc.vector.memset(h0[:], 0.0)
        nc.vector.memset(hT0[:], 0.0)

        # Pre-consume ident on PE: transpose-mode matmuls lower to the LDW
        # struct which holds only ONE sync wait; without this the first real
        # transpose would need both its data wait and the ident-DMA wait.
        warm = ps_rT.tile([128, 128], F32, tag="rT")
        nc.tensor.transpose(warm[:], ident_sb[:], ident_sb[:])

        # HAM warm-up: ~5us of back-to-back matmuls so the PE clock gate
        # opens (K=8/8). Steady-state PE idle gaps stay below the ~3.4us
        # MID window, so the array never re-throttles afterwards.
        wps = ps_hT.tile([128, 512], F32, tag="pT")
        for i in range(26):
            nc.tensor.matmul(
                wps[:, 0:512],
                lhsT=wh_ur_sb[:, 0, 0, 0:128],
                rhs=wh_ur_sb[:, 0, :, :].rearrange("p a b -> p (a b)")[:, 0:512],
                start=(i == 0),
                stop=(i == 25),
                skip_group_check=True,
            )

        def emit_gx(t):
            """Input-projection matmuls for step t -> fresh psum tiles."""
            pu = ps_ur.tile([128, 256], F32)
            ph = ps_hh.tile([128, 128], F32)
            # Broadcast-init both tiles (K=1 matmul of a zeros row): zeroes
            # every partition incl. unused lanes, so downstream full-width
            # reads are always on initialized data.
            nc.tensor.matmul(
                pu[:, :],
                lhsT=ones_sb[:],
                rhs=zrow_sb[:, 0:256],
                start=True,
                stop=False,
                skip_group_check=True,
            )
            nc.tensor.matmul(
                ph[:, :],
                lhsT=ones_sb[:],
                rhs=zrow_sb[:, 256:384],
                start=True,
                stop=False,
                skip_group_check=True,
            )
            if with_bias:
                for g in range(4):
                    nc.tensor.matmul(
                        pu[32 * g : 32 * g + 16, :],
                        lhsT=ones_sb[:, 0:16],
                        rhs=bias_ur_sb[:, g, :],
                        start=False,
                        stop=False,
                        tile_position=(0, 32 * g),
                        skip_group_check=True,
                    )
                    nc.tensor.matmul(
                        ph[32 * g : 32 * g + 16, :],
                        lhsT=ones_sb[:, 0:16],
                        rhs=bias_hh_sb[:, g, :],
                        start=False,
                        stop=False,
                        tile_position=(0, 32 * g),
                        skip_group_check=True,
                    )
            for kc in range(KC_X):
                for g in range(4):
                    st = xT_sb[:, kc, t, :]
                    nc.tensor.matmul(
                        pu[32 * g : 32 * g + 16, :],
                        lhsT=st,
                        rhs=wx_sb[:, kc, g, 0:256],
                        start=False,
                        stop=False,
                        tile_position=(0, 32 * g),
                        skip_group_check=True,
                    )
                    nc.tensor.matmul(
                        ph[32 * g : 32 * g + 16, :],
                        lhsT=st,
                        rhs=wx_sb[:, kc, g, 256:384],
                        start=False,
                        stop=False,
                        tile_position=(0, 32 * g),
                        skip_group_check=True,
                    )
            return pu, ph

        pu_cur, ph_cur = emit_gx(0)

        h_prev = h0  # f32 packed [128,128] (AP-able)
        h_prev_ap = h0[:]
        hT_prev = hT0
        stage_cur = None

        for t in range(t_steps):
            if t % OUT_BLOCK == 0:
                stage_cur = stage_pool.tile([128, OUT_BLOCK, 128], F32)

            # ---- gx for t+1 (fills PE gaps while step-t chain runs) ----
            if t + 1 < t_steps:
                pu_nxt, ph_nxt = emit_gx(t + 1)
            else:
                pu_nxt = ph_nxt = None

            # ---- recurrent matmuls: gates u,r ----
            for kc in range(KC_H):
                for g in range(4):
                    nc.tensor.matmul(
                        pu_cur[32 * g : 32 * g + 16, :],
                        lhsT=hT_prev[:, 32 * kc : 32 * kc + 16],
                        rhs=wh_ur_sb[:, kc, g, :],
                        start=False,
                        stop=(kc == KC_H - 1) and (g == 3),
                        tile_position=(0, 32 * g),
                        skip_group_check=True,
                    )

            # ---- sigmoid(r), sigmoid(u) ----
            r_s = gates.tile([128, 128], BF16, tag="r_s")
            u_s = gates.tile([128, 128], F32, tag="u_s")
            nc.scalar.activation(
                r_s[:], pu_cur[:, 128:256], mybir.ActivationFunctionType.Sigmoid
            )
            nc.scalar.activation(
                u_s[:], pu_cur[:, 0:128], mybir.ActivationFunctionType.Sigmoid
            )

            # ---- transpose r (bf16), tT = rT * hT ----
            rT = ps_rT.tile([128, 128], BF16, tag="rT")
            nc.tensor.transpose(rT[:], r_s[:], ident16_sb[:])
            tT = tT_pool.tile([128, 128], BF16)
            nc.vector.tensor_mul(tT[:], rT[:], hT_prev[:])

            # ---- candidate matmul ----
            for kc in range(KC_H):
                for g in range(4):
                    nc.tensor.matmul(
                        ph_cur[32 * g : 32 * g + 16, :],
                        lhsT=tT[:, 32 * kc : 32 * kc + 16],
                        rhs=wh_hh_sb[:, kc, g, :],
                        start=False,
                        stop=(kc == KC_H - 1) and (g == 3),
                        tile_position=(0, 32 * g),
                        skip_group_check=True,
                    )

            # ---- tanh, blend: h_new = u*hh + (1-u)*h = u*hh - (u-1)*h ----
            hh_s = gates.tile([128, 128], F32, tag="hh_s")
            nc.scalar.activation(
                hh_s[:], ph_cur[:], mybir.ActivationFunctionType.Tanh
            )
            w_s = gates.tile([128, 128], F32, tag="w_s")
            nc.vector.scalar_tensor_tensor(
                w_s[:],
                in0=u_s[:],
                scalar=1.0,
                in1=h_prev_ap,
                op0=mybir.AluOpType.subtract,
                op1=mybir.AluOpType.mult,
            )
            v_s = gates.tile([128, 128], F32, tag="v_s")
            nc.vector.tensor_mul(v_s[:], u_s[:], hh_s[:])
            h_new_ap = stage_cur[:, t % OUT_BLOCK, :]
            nc.vector.tensor_sub(h_new_ap, v_s[:], w_s[:])

            # ---- transpose h_new -> hT (bf16) for next step ----
            hT_new = hT_pool.tile([128, 128], BF16)
            pT = ps_hT.tile([128, 128], F32)
            nc.tensor.transpose(pT[:], h_new_ap, ident_sb[:])
            nc.scalar.copy(out=hT_new[:], in_=pT[:])

            # ---- flush output block ----
            if (t + 1) % OUT_BLOCK == 0 or t == t_steps - 1:
                n = t % OUT_BLOCK + 1
                t0 = t - n + 1
                nc.sync.dma_start(
                    out=out[:, t0 : t0 + n, :],
                    in_=stage_cur[:, 0:n, :],
                )

            h_prev = stage_cur
            h_prev_ap = h_new_ap
            hT_prev = hT_new
            pu_cur, ph_cur = pu_nxt, ph_nxt

    # Bacc.finalize runs the compile pipeline (wait splitting, register
    # allocation). run_bass_via_pjrt serializes nc as-is, so do it here.
    nc.finalize()
    return nc


# ---------------------------------------------------------------------------
# Host-side packing / unpacking
# ---------------------------------------------------------------------------


def _pack_x(xc):
    """xc [BL, t, 256] (already direction-adjusted) -> [128, 2, t, BL] bf16."""
    t = xc.shape[1]
    a = np.ascontiguousarray(xc.transpose(2, 1, 0))  # [256, t, BL]
    a = a.reshape(KC_X, 128, t, BL).transpose(1, 0, 2, 3)
    return np.ascontiguousarray(a).astype(bfloat16)


def _pack_wh(wh):
    u = wh[:, 0:512].reshape(512, 4, 128)
    r = wh[:, 512:1024].reshape(512, 4, 128)
    hh = wh[:, 1024:1536].reshape(512, 4, 128)
    ur = np.concatenate([u, r], axis=2)  # [512, 4, 256]
    ur = ur.reshape(KC_H, 128, 4, 256).transpose(1, 0, 2, 3)
    hh = hh.reshape(KC_H, 128, 4, 128).transpose(1, 0, 2, 3)
    return (
        np.ascontiguousarray(ur).astype(bfloat16),
        np.ascontiguousarray(hh).astype(bfloat16),
    )


def _pack_wx(wx):
    u = wx[:, 0:512].reshape(256, 4, 128)
    r = wx[:, 512:1024].reshape(256, 4, 128)
    hh = wx[:, 1024:1536].reshape(256, 4, 128)
    a = np.concatenate([u, r, hh], axis=2)  # [256, 4, 384]
    a = a.reshape(KC_X, 128, 4, 384).transpose(1, 0, 2, 3)
    return np.ascontiguousarray(a).astype(bfloat16)


def _pack_bias(b):
    u = b[0:512].reshape(4, 128)
    r = b[512:1024].reshape(4, 128)
    hh = b[1024:1536].reshape(4, 128)
    ur = np.concatenate([u, r], axis=1)[None].astype(np.float32)  # [1, 4, 256]
    return np.ascontiguousarray(ur), np.ascontiguousarray(
        hh[None].astype(np.float32)
    )


def make_in_maps(x, Wx_f, Wh_f, b_f, Wx_b, Wh_b, b_b, t_steps=T, with_bias=False):
    x = np.asarray(x, dtype=np.float32)
    ident = np.eye(128, dtype=np.float32)
    per_dir = {}
    for d, (wx, wh, bb) in enumerate(
        [(Wx_f, Wh_f, b_f), (Wx_b, Wh_b, b_b)]
    ):
        ur, hh = _pack_wh(np.asarray(wh, np.float32))
        wxp = _pack_wx(np.asarray(wx, np.float32))
        ent = {"wh_ur": ur, "wh_hh": hh, "wx_all": wxp}
        if with_bias:
            bur, bhh = _pack_bias(np.asarray(bb, np.float32))
            ent.update(bias_ur=bur, bias_hh=bhh)
        per_dir[d] = ent

    in_maps = []
    for c in range(NCORES):
        d, q = divmod(c, 4)
        xc = x[q * BL : (q + 1) * BL, :t_steps]
        if d == 1:
            xc = xc[:, ::-1]
        m = {
            "xT": _pack_x(xc),
            "ident": ident,
            "ident16": ident.astype(bfloat16),
            "zrow": np.zeros((1, 384), dtype=bfloat16),
            "ones_row": np.ones((1, 128), dtype=bfloat16),
        }
        m.update(per_dir[d])
        in_maps.append(m)
    return in_maps


def unpack_outputs(results, t_steps=T):
    out = np.empty((B, t_steps, 2 * U), dtype=np.float32)
    for c in range(NCORES):
        d, q = divmod(c, 4)
        r = results[c]["out_packed"].reshape(4, 32, t_steps, 128)[:, 0:BL]
        out[q * BL : (q + 1) * BL, :, d * U : (d + 1) * U] = (
            r.transpose(1, 2, 0, 3).reshape(BL, t_steps, U)
        )
    return out


_CACHE = {}


def kernel(x, Wx_f, Wh_f, b_f, Wx_b, Wh_b, b_b):
    with_bias = bool(
        np.any(np.asarray(b_f) != 0) or np.any(np.asarray(b_b) != 0)
    )
    key = ("prog", T, with_bias)
    if key not in _CACHE:
        _CACHE[key] = build_program(T, with_bias)
    nc = _CACHE[key]
    in_maps = make_in_maps(
        x, Wx_f, Wh_f, b_f, Wx_b, Wh_b, b_b, T, with_bias
    )
    res = run_bass_kernel_spmd(nc, in_maps, list(range(NCORES)))
    return unpack_outputs(res.results, T)


if __name__ == "__main__":
    mode = sys.argv[1] if len(sys.argv) > 1 else "sim"
    if mode == "sim":
        # Small-T single-core simulation vs numpy GRU.
        ts = int(sys.argv[2]) if len(sys.argv) > 2 else 8
        rng = np.random.default_rng(0)
        x = rng.standard_normal((B, ts, D), dtype=np.float32)
        Wx = (rng.standard_normal((D, 3 * U), dtype=np.float32) / np.sqrt(D)).astype(
            np.float32
        )
        Wh = (rng.standard_normal((U, 3 * U), dtype=np.float32) / np.sqrt(U)).astype(
            np.float32
        )
        bz = np.zeros(3 * U, np.float32)

        nc = build_program(ts, with_bias=False)
        in_maps = make_in_maps(x, Wx, Wh, bz, Wx, Wh, bz, ts, False)

        from concourse.bass_interp import MultiCoreSim

        sim = MultiCoreSim(nc, 1)
        for k, v in in_maps[0].items():
            sim.cores[0].tensor(k)[:] = v
        sim.simulate()
        got = sim.cores[0].tensor("out_packed")  # [128, ts, 128]
        got = (
            got.reshape(4, 32, ts, 128)[:, 0:BL]
            .transpose(1, 2, 0, 3)
            .reshape(BL, ts, U)
        )

        # numpy reference GRU (forward, batch quarter 0)
        h = np.zeros((BL, U), np.float32)
        exp = np.zeros((BL, ts, U), np.float32)
        xs = x[0:BL, :ts].astype(np.float32)
        for t in range(ts):
            gx = xs[:, t] @ Wx
            pu = gx[:, :U] + h @ Wh[:, :U]
            pr = gx[:, U : 2 * U] + h @ Wh[:, U : 2 * U]
            u = 1 / (1 + np.exp(-pu))
            r = 1 / (1 + np.exp(-pr))
            hh = np.tanh(gx[:, 2 * U :] + (r * h) @ Wh[:, 2 * U :])
            h = (1 - u) * h + u * hh
            exp[:, t] = h
        err = np.abs(got - exp)
        denom = max(1e-6, np.abs(exp).max())
        print("max abs err:", err.max(), " rel:", err.max() / denom)
        print("sample got:", got[0, -1, :4], " exp:", exp[0, -1, :4])
    else:
        print("unknown mode", mode)
